# revision 16
# baseline (speedup 1.0000x reference)
"""Trainium2 Bass kernel for nn_AttentionBranch (sparse GQA attention + RoPE).

Problem (hardcoded): B=1, S=2176, 32 q heads, 8 kv heads, head_dim=128,
mask = causal & (sliding-window-256 | kv < 128 meta prefix), fp32 io.

Sharding: 8 cores; core c owns q heads [4c, 4c+4) and kv head c (GQA group).

Per-core dataflow (SPMD, one Bass program):
  - RoPE on-device from host-shipped straight + half-swapped q/k copies and
    cos / sign-folded-sin tables (3 DVE ops per tensor).
  - Block-sparse attention over 128-row q blocks: kv blocks {0, i-2, i-1, i}.
    Work is organized in 3-q-block pieces; per piece all QK scores (<=1536
    cols, kv on partitions) go into ONE 3-bank PSUM group tile via QK
    matmuls + triangular additive masks (matmul accumulate), then ONE exp
    (ACT) per piece -> pb bf16 (6 ACT calls/head amortize the ~290ns
    ACTIVATE fixed cost).
  - PV+Z fused: V ships with a ones column appended ([kv, NB, 129]); one
    probs-stationary matmul per (strip, 128-col q sub-block) produces
    out[q, dv 0..127] AND the softmax denominator Z in column 128 of the
    same PSUM accumulator -- the old ones-matmul row-sum pass (29% of PE
    cycles) collapses into 1 extra rhs column per block.
  - Normalize per piece: strided reciprocal of the Z columns + one
    broadcast tensor_mul; output written bf16 [q-block, q, dv] (host
    widens to f32 and reshapes).
  - Pieces are pipelined with a 2-piece lag (PE runs QK of pieces g+1/g+2
    while ACT runs exp of piece g), PSUM: 2x3-bank group tiles + 2x1-bank
    out accumulators = 8 banks.
"""

import math
import os
from functools import lru_cache

import numpy as np
import ml_dtypes

S = 2176
D = 128
NB = S // 128  # 17 q/kv blocks
HQ_PER_CORE = 4
N_CORES = 8
WINDOW = 256
META = 128
ROPE_BASE = 10000.0
SCALE = 1.0 / math.sqrt(D)

BF16 = ml_dtypes.bfloat16
LAST_RESULT = None

# q-block ranges of the per-head processing pieces. 3 blocks/piece so the
# piece's out+Z accumulator (3 x 129 f32 = 387 cols) fits ONE PSUM bank.
PIECES = [(0, 2), (3, 5), (6, 8), (9, 11), (12, 14), (15, 16)]


def _strips_for_piece(b0, b1):
    """Work list for q-blocks [b0, b1]. Each strip is one kv-block (or meta
    chunk) x a contiguous span of q columns."""
    strips = []
    lo_col = b0 * 128
    hi_col = (b1 + 1) * 128
    # meta chunk: kv block 0, dense except causal diag for q-block 0.
    col = lo_col
    while col < hi_col:
        span = min(512, hi_col - col)
        strips.append(
            dict(
                kvblk=0,
                qlo=col,
                qhi=col + span,
                meta=True,
                diag_u=0 if col == 0 else None,
                i2_u=None,
            )
        )
        col += span
    # window strips: kv block j covers q blocks {j, j+1, j+2} (j >= 1).
    for j in range(1, NB):
        i0 = max(j, b0)
        i1 = min(j + 2, b1)
        if i0 > i1:
            continue
        strips.append(
            dict(
                kvblk=j,
                qlo=i0 * 128,
                qhi=(i1 + 1) * 128,
                meta=False,
                diag_u=0 if i0 == j else None,
                i2_u=(i1 - i0) * 128 if i1 == j + 2 else None,
            )
        )
    return strips


def _pack_group(strips):
    """Hole-free packing of a piece's strips into one <=1536-col group tile
    such that every strip stays inside one 512-col PSUM bank. Sets
    st['goff']; returns total cols. Meta strip is packed (and emitted)
    first so its PV+Z matmuls open each q-block's accumulation chain."""

    def span(s):
        return s["qhi"] - s["qlo"]

    rest = sorted(strips, key=lambda s: (-span(s), not s["meta"]))
    order, fill = [], 0
    while rest:
        pick = None
        for s in rest:
            sp = span(s)
            if fill + sp <= 1536 and (fill % 512) + sp <= 512:
                pick = s
                break
        assert pick is not None, "packing stuck"
        rest.remove(pick)
        pick["goff"] = fill
        fill += span(pick)
        order.append(pick)
    assert order[0]["meta"]
    return order, fill


@lru_cache(maxsize=1)
def _build_program():
    import concourse.bass as bass
    import concourse.mybir as mybir
    import concourse.tile as tile
    from concourse import bacc

    bf = mybir.dt.bfloat16
    f32 = mybir.dt.float32
    EXP = mybir.ActivationFunctionType.Exp

    nc = bacc.Bacc(None)

    q0_d = nc.declare_dram_parameter("q0", [2, D, S], bf, isOutput=False)
    qr_d = nc.declare_dram_parameter("qr", [3, D, S], bf, isOutput=False)
    kt_d = nc.declare_dram_parameter("kt", [2, D, S], bf, isOutput=False)
    v_d = nc.declare_dram_parameter("v", [D, NB, 129], bf, isOutput=False)
    cs_d = nc.declare_dram_parameter("cs", [2, 64, S], bf, isOutput=False)
    msk_d = nc.declare_dram_parameter("msk", [D, 3, 128], bf, isOutput=False)
    out_d = nc.declare_dram_parameter("out", [HQ_PER_CORE, NB, D, 128], bf, isOutput=True)

    with tile.TileContext(nc) as tc:
        with (
            tc.tile_pool(name="persist", bufs=1) as persist,
            tc.tile_pool(name="probs", bufs=3) as probs_pool,
            tc.tile_pool(name="norm", bufs=3) as norm_pool,
            tc.tile_pool(name="osb", bufs=3) as osb_pool,
            tc.tile_pool(name="grp", bufs=2, space="PSUM") as grp_psum,
            tc.tile_pool(name="acc", bufs=2, space="PSUM") as acc_psum,
        ):
            qt0 = persist.tile([D, 2, S], bf)
            qr = persist.tile([D, 3, S], bf)
            qsw = persist.tile([D, 3, S], bf)  # DVE-swapped q heads 1-3
            kt = persist.tile([D, 2, S], bf)
            vt = persist.tile([D, NB, 129], bf)
            cs = persist.tile([D, 2, S], bf)
            msk = persist.tile([D, 3, 128], bf)
            ones = persist.tile([D, 128], bf)
            ropek = persist.tile([D, S], bf)
            ropeq = persist.tile([D, HQ_PER_CORE, S], bf)
            ropet = persist.tile([D, 2, S], bf)

            # Host ships straight + half-swapped copies of q/k; chunk
            # boundaries match piece needs so head-0 attention starts as
            # early as possible while later heads stream in.
            ktr = kt_d.rearrange("s d t -> d s t")
            csr = cs_d.rearrange("s d t -> d s t")
            q0r = q0_d.rearrange("s d t -> d s t")
            # Chunked so head-0 rope starts early; q heads 1-3 ship single
            # copy (their RoPE half-swap happens on DVE) and land before the
            # head-0 tail chunks they do not gate.
            chunks = [(0, 512), (512, 1024), (1024, 1536), (1536, S)]
            for ci, (lo, hi) in enumerate(chunks):
                nc.sync.dma_start(out=kt[:, :, lo:hi], in_=ktr[:, :, lo:hi])
                nc.sync.dma_start(out=cs[0:64, :, lo:hi], in_=csr[:, :, lo:hi])
                nc.sync.dma_start(out=qt0[:, :, lo:hi], in_=q0r[:, :, lo:hi])
                if ci == 0:
                    nc.sync.dma_start(out=msk, in_=msk_d[:])
                elif ci == 1:
                    nc.sync.dma_start(out=vt, in_=v_d[:])
                    nc.sync.dma_start(out=qr[:, 0], in_=qr_d[0])
                elif ci == 3:
                    nc.sync.dma_start(out=qr[:, 1], in_=qr_d[1])
                    nc.sync.dma_start(out=qr[:, 2], in_=qr_d[2])
            nc.vector.memset(ones, 1.0)

            # trigger the exp ACT-table load early (off the critical path)
            tldw = norm_pool.tile([D, 3, 1], f32, tag="rz")
            nc.scalar.activation(tldw[:, 0], ones[:, :1], EXP)

            # PE warm-up: DMA-independent dummy matmuls (ones x ones) keep
            # the HAM activity window busy so the real stream starts ramped.
            wz = acc_psum.tile([D, 512], f32, tag="ot")
            for _ in range(34):
                nc.tensor.matmul(
                    wz[:, :128], lhsT=ones, rhs=ones, start=True, stop=True
                )

            def rope_k(lo, hi):
                sl = slice(lo, hi)
                # rebuild the cos/sin upper halves (cos repeats; sinpm upper
                # is the negation of the shipped lower half) - exact in bf16
                nc.vector.tensor_copy(cs[64:128, 0, sl], cs[0:64, 0, sl])
                nc.vector.tensor_scalar_mul(cs[64:128, 1, sl], cs[0:64, 1, sl], -1.0)
                nc.vector.tensor_mul(ropek[:, sl], kt[:, 0, sl], cs[:, 0, sl])
                nc.vector.tensor_mul(ropet[:, 0, sl], kt[:, 1, sl], cs[:, 1, sl])
                nc.vector.tensor_add(ropek[:, sl], ropek[:, sl], ropet[:, 0, sl])

            def rope_q(h, lo, hi):
                sl = slice(lo, hi)
                straight = qt0[:, 0, sl] if h == 0 else qr[:, h - 1, sl]
                swapped = qt0[:, 1, sl] if h == 0 else qsw[:, h - 1, sl]
                nc.vector.tensor_mul(ropeq[:, h, sl], straight, cs[:, 0, sl])
                nc.vector.tensor_mul(ropet[:, 1, sl], swapped, cs[:, 1, sl])
                nc.vector.tensor_add(
                    ropeq[:, h, sl], ropeq[:, h, sl], ropet[:, 1, sl]
                )

            def swap_q(h):
                # full-span partition-offset copies (4x single-src DVE mode)
                nc.vector.tensor_copy(qsw[0:64, h - 1], qr[64:128, h - 1])
                nc.vector.tensor_copy(qsw[64:128, h - 1], qr[0:64, h - 1])

            def emit_qk(h, st, gp):
                """QK + additive-mask matmuls for one strip into the group
                tile at st['goff']."""
                span = st["qhi"] - st["qlo"]
                go = st["goff"]
                masks = []
                if st["diag_u"] is not None:
                    masks.append((st["diag_u"], 0))
                if st["i2_u"] is not None:
                    masks.append((st["i2_u"], 1))
                nc.tensor.matmul(
                    gp[:, go : go + span],
                    lhsT=ropek[:, st["kvblk"] * 128 : (st["kvblk"] + 1) * 128],
                    rhs=ropeq[:, h, st["qlo"] : st["qhi"]],
                    start=True,
                    stop=not masks,
                )
                for mi, (u, g) in enumerate(masks):
                    nc.tensor.matmul(
                        gp[:, go + u : go + u + 128],
                        lhsT=msk[:, 2],
                        rhs=msk[:, g],
                        start=False,
                        stop=mi == len(masks) - 1,
                    )

            def emit_piece_back(work):
                """PV+Z matmuls for all strips of a piece, then normalize +
                store. otq layout: q-block b of the piece at cols
                [129b, 129b+129); col 129b+128 accumulates Z."""
                h, b0, b1, order, pbg, otq = work
                nq = b1 - b0 + 1
                last_for_qb = {}
                for si, st in enumerate(order):
                    for qb in range(st["qlo"] // 128, st["qhi"] // 128):
                        last_for_qb[qb] = si
                # start=True ONLY on the very first matmul into the bank: it
                # clears has_written for the WHOLE bank, so later first-
                # writes per q-block chain must use start=False (overwrite-
                # where-bit-unset initializes them correctly).
                for si, st in enumerate(order):
                    for k in range((st["qhi"] - st["qlo"]) // 128):
                        qb = st["qlo"] // 128 + k
                        b = qb - b0
                        nc.tensor.matmul(
                            otq[:, b],
                            lhsT=pbg[:, st["goff"] + 128 * k : st["goff"] + 128 * (k + 1)],
                            rhs=vt[:, st["kvblk"]],
                            start=si == 0 and k == 0,
                            stop=last_for_qb[qb] == si,
                            skip_group_check=True,
                        )
                # normalize: strided recip of the nq Z columns, then one
                # dv-broadcast multiply; store bf16.
                rzt = norm_pool.tile([D, 3, 1], f32, tag="rz")
                nc.vector.reciprocal_approx_fast(
                    rzt[:, :nq], otq[:, :nq, 128:129]
                )
                osb = osb_pool.tile([D, 3, 128], bf, tag="osb")
                nc.vector.tensor_mul(
                    osb[:, :nq],
                    otq[:, :nq, :128],
                    rzt[:, :nq].broadcast_to([D, nq, 128]),
                )
                orh = out_d[h].rearrange("j p v -> p j v")
                nc.sync.dma_start(out=orh[:, b0 : b0 + nq], in_=osb[:, :nq])

            # Software-pipelined emission with a lag: PE runs QK of later
            # pieces while ACT computes earlier pieces' exps. Units are
            # ordered by input-DMA arrival (head-0 chunks first, head 1
            # interleaved as its q lands, heads 2/3 after) so the exp
            # stream runs dense from the first piece instead of pacing on
            # head-0's chunk arrivals. Rope/swap ops are emitted just in
            # time, keeping the DVE queue free of long head-of-line waits.
            from collections import deque

            order = [(h, p) for h in range(HQ_PER_CORE) for p in range(6)]
            ropeq_done = [0] * HQ_PER_CORE
            ropek_done = 0
            swapped = set()
            pending = deque()
            for ui, (h, pidx) in enumerate(order):
                b0, b1 = PIECES[pidx]
                need = (b1 + 1) * 128
                if h >= 1 and h not in swapped:
                    swap_q(h)
                    swapped.add(h)
                if ropek_done < need:
                    rope_k(ropek_done, need)
                    ropek_done = need
                if ropeq_done[h] < need:
                    rope_q(h, ropeq_done[h], need)
                    ropeq_done[h] = need
                LAG = 1 if ui < 4 else 2
                strips, gcols = _pack_group(_strips_for_piece(b0, b1))
                gp = grp_psum.tile([D, 1536], f32, tag="gp")
                pbg = probs_pool.tile([D, 1536], bf, tag="pb")
                otq = acc_psum.tile([D, 3, 129], f32, tag="ot")
                for st in strips:
                    emit_qk(h, st, gp)
                nc.scalar.activation(
                    pbg[:, :gcols], gp[:, :gcols], EXP, scale=SCALE
                )
                pending.append((h, b0, b1, strips, pbg, otq))
                while len(pending) > LAG:
                    emit_piece_back(pending.popleft())
            while pending:
                emit_piece_back(pending.popleft())

    nc.finalize()
    return nc


@lru_cache(maxsize=1)
def _rope_tables():
    inv_freq = 1.0 / (ROPE_BASE ** (np.arange(0, D, 2, dtype=np.float64) / D))
    pos = np.arange(S, dtype=np.float64)
    freqs = pos[:, None] * inv_freq[None, :]  # [S, 64]
    emb = np.concatenate([freqs, freqs], axis=-1)  # [S, D]
    # match the f32 reference: compute cos/sin at f32 granularity
    cosT = np.cos(emb.astype(np.float32)).T.astype(np.float32)  # [D, S]
    sinT = np.sin(emb.astype(np.float32)).T.astype(np.float32)
    sinTpm = np.concatenate([-sinT[:64], sinT[64:]], axis=0)
    return cosT, sinTpm


def _mask_tiles():
    """[128, 3, 128]: additive score masks (0 keep / -1e30 drop) for the
    causal-diag and window-tail blocks, plus a 128x128 identity (the
    stationary operand of the mask-accumulate matmuls)."""
    c = np.arange(128)[:, None]
    u = np.arange(128)[None, :]
    a_diag = np.where(u >= c, 0.0, -1e30).astype(np.float32)
    a_tail = np.where(u <= c, 0.0, -1e30).astype(np.float32)
    ident = np.eye(128, dtype=np.float32)
    return np.stack([a_diag, a_tail, ident], axis=1)  # [128, 3, 128]


def _swap_halves(xT):
    return np.concatenate([xT[64:], xT[:64]], axis=0)


def _install_ntff_shim():
    """Provide antenv.axon_hooks (NTFF profile hook) if the image lacks it,
    so run_bass_kernel_spmd(trace=True) can capture HW profiles via the
    axon PJRT .so. Silently no-ops if unavailable."""
    import sys
    import types

    try:
        from antenv.axon_hooks import get_axon_ntff_profile_hook  # noqa: F401

        return
    except ImportError:
        pass
    try:
        import contextlib
        import ctypes

        lib = ctypes.CDLL("/opt/axon/libaxon_pjrt.so")
        if not hasattr(lib, "axon_start_nrt_profile"):
            return
        lib.axon_start_nrt_profile.argtypes = [
            ctypes.POINTER(ctypes.c_int64),
            ctypes.c_size_t,
        ]
        lib.axon_start_nrt_profile.restype = ctypes.c_int64
        lib.axon_stop_nrt_profile.argtypes = [ctypes.c_char_p]
        lib.axon_stop_nrt_profile.restype = ctypes.c_int64

        @contextlib.contextmanager
        def _hook(output_dir, device_ids):
            import jax

            jax.devices()
            if device_ids:
                ids = (ctypes.c_int64 * len(device_ids))(*device_ids)
                rc = lib.axon_start_nrt_profile(ids, len(device_ids))
            else:
                rc = lib.axon_start_nrt_profile(None, 0)
            if rc != 0:
                raise RuntimeError(f"axon_start_nrt_profile rc={rc}")
            try:
                yield
            finally:
                n = lib.axon_stop_nrt_profile(str(output_dir).encode())
                print(f"ntff profile: {n} file(s) -> {output_dir}", file=sys.stderr)

        mod = types.ModuleType("antenv.axon_hooks")
        mod._hook = _hook
        mod.get_axon_ntff_profile_hook = lambda: _hook
        mod.set_axon_ntff_profile_hook = lambda h: setattr(mod, "_hook", h)
        import antenv

        antenv.axon_hooks = mod
        sys.modules["antenv.axon_hooks"] = mod
    except Exception:
        pass


def kernel(query_states, key_states, value_states):
    from concourse.bass_utils import run_bass_kernel_spmd

    _install_ntff_shim()

    nc = _build_program()

    q = np.asarray(query_states)[0]  # [S, 4096]
    k = np.asarray(key_states)[0]  # [S, 1024]
    v = np.asarray(value_states)[0]  # [S, 1024]

    cosT, sinTpm = _rope_tables()
    cs = np.stack([cosT[:64], sinTpm[:64]], axis=0).astype(BF16)  # [2, 64, S]
    msk = _mask_tiles().astype(BF16)

    in_maps = []
    for c in range(N_CORES):
        q0h = np.ascontiguousarray(q[:, 4 * c * D : (4 * c + 1) * D].T)  # [D, S]
        q0 = np.stack([q0h, _swap_halves(q0h)], axis=0).astype(BF16)
        qr = np.empty((3, D, S), dtype=BF16)
        for hh in range(1, HQ_PER_CORE):
            h = 4 * c + hh
            qr[hh - 1] = np.ascontiguousarray(q[:, h * D : (h + 1) * D].T).astype(BF16)
        kh = np.ascontiguousarray(k[:, c * D : (c + 1) * D].T)
        kt = np.stack([kh, _swap_halves(kh)], axis=0).astype(BF16)
        vh = v[:, c * D : (c + 1) * D]  # [S, D]
        vts = np.ones((D, NB, 129), dtype=BF16)
        vts[:, :, :128] = vh.reshape(NB, 128, D).transpose(1, 0, 2).astype(BF16)
        in_maps.append({"q0": q0, "qr": qr, "kt": kt, "v": vts, "cs": cs, "msk": msk})

    res = run_bass_kernel_spmd(nc, in_maps, core_ids=list(range(N_CORES)))
    global LAST_RESULT
    LAST_RESULT = res

    out = np.empty((S, 32, D), dtype=np.float32)
    for c in range(N_CORES):
        o = np.asarray(res.results[c]["out"], dtype=np.float32)  # [4, NB, D, 128]
        for hh in range(HQ_PER_CORE):
            out[:, 4 * c + hh, :] = o[hh].reshape(S, D)
    return out.reshape(1, S, 32 * D)


# revision 17
# speedup vs baseline: 1.0750x; 1.0750x over previous
"""Trainium2 Bass kernel for nn_AttentionBranch (sparse GQA attention + RoPE).

Problem (hardcoded): B=1, S=2176, 32 q heads, 8 kv heads, head_dim=128,
mask = causal & (sliding-window-256 | kv < 128 meta prefix), fp32 io.

Sharding: 8 cores; core c owns q heads [4c, 4c+4) and kv head c (GQA group).

Per-core dataflow (SPMD, one Bass program):
  - RoPE on-device from host-shipped straight + half-swapped q/k copies and
    cos / sign-folded-sin tables (3 DVE ops per tensor).
  - Block-sparse attention over 128-row q blocks: kv blocks {0, i-2, i-1, i}.
    Work is organized in 3-q-block pieces; per piece all QK scores (<=1536
    cols, kv on partitions) go into ONE 3-bank PSUM group tile via QK
    matmuls + triangular additive masks (matmul accumulate), then ONE exp
    (ACT) per piece -> pb bf16 (6 ACT calls/head amortize the ~290ns
    ACTIVATE fixed cost).
  - PV+Z fused: V ships with a ones column appended ([kv, NB, 129]); one
    probs-stationary matmul per (strip, 128-col q sub-block) produces
    out[q, dv 0..127] AND the softmax denominator Z in column 128 of the
    same PSUM accumulator -- the old ones-matmul row-sum pass (29% of PE
    cycles) collapses into 1 extra rhs column per block.
  - Normalize per piece: strided reciprocal of the Z columns + one
    broadcast tensor_mul; output written bf16 [q-block, q, dv] (host
    widens to f32 and reshapes).
  - Pieces are pipelined with a 2-piece lag (PE runs QK of pieces g+1/g+2
    while ACT runs exp of piece g), PSUM: 2x3-bank group tiles + 2x1-bank
    out accumulators = 8 banks.
"""

import math
import os
from functools import lru_cache

import numpy as np
import ml_dtypes

S = 2176
D = 128
NB = S // 128  # 17 q/kv blocks
HQ_PER_CORE = 4
N_CORES = 8
WINDOW = 256
META = 128
ROPE_BASE = 10000.0
SCALE = 1.0 / math.sqrt(D)

BF16 = ml_dtypes.bfloat16
LAST_RESULT = None

# q-block ranges of the per-head processing pieces. 3 blocks/piece so the
# piece's out+Z accumulator (3 x 129 f32 = 387 cols) fits ONE PSUM bank.
PIECES = [(0, 2), (3, 5), (6, 8), (9, 11), (12, 14), (15, 16)]


def _strips_for_piece(b0, b1):
    """Work list for q-blocks [b0, b1]. Each strip is one kv-block (or meta
    chunk) x a contiguous span of q columns."""
    strips = []
    lo_col = b0 * 128
    hi_col = (b1 + 1) * 128
    # meta chunk: kv block 0, dense except causal diag for q-block 0.
    col = lo_col
    while col < hi_col:
        span = min(512, hi_col - col)
        strips.append(
            dict(
                kvblk=0,
                qlo=col,
                qhi=col + span,
                meta=True,
                diag_u=0 if col == 0 else None,
                i2_u=None,
            )
        )
        col += span
    # window strips: kv block j covers q blocks {j, j+1, j+2} (j >= 1).
    for j in range(1, NB):
        i0 = max(j, b0)
        i1 = min(j + 2, b1)
        if i0 > i1:
            continue
        strips.append(
            dict(
                kvblk=j,
                qlo=i0 * 128,
                qhi=(i1 + 1) * 128,
                meta=False,
                diag_u=0 if i0 == j else None,
                i2_u=(i1 - i0) * 128 if i1 == j + 2 else None,
            )
        )
    return strips


def _pack_group(strips):
    """Hole-free packing of a piece's strips into one <=1536-col group tile
    such that every strip stays inside one 512-col PSUM bank. Sets
    st['goff']; returns total cols. Meta strip is packed (and emitted)
    first so its PV+Z matmuls open each q-block's accumulation chain."""

    def span(s):
        return s["qhi"] - s["qlo"]

    rest = sorted(strips, key=lambda s: (-span(s), not s["meta"]))
    order, fill = [], 0
    while rest:
        pick = None
        for s in rest:
            sp = span(s)
            if fill + sp <= 1536 and (fill % 512) + sp <= 512:
                pick = s
                break
        assert pick is not None, "packing stuck"
        rest.remove(pick)
        pick["goff"] = fill
        fill += span(pick)
        order.append(pick)
    assert order[0]["meta"]
    return order, fill


@lru_cache(maxsize=1)
def _build_program():
    import concourse.bass as bass
    import concourse.mybir as mybir
    import concourse.tile as tile
    from concourse import bacc

    bf = mybir.dt.bfloat16
    f32 = mybir.dt.float32
    EXP = mybir.ActivationFunctionType.Exp

    nc = bacc.Bacc(None)

    q0_d = nc.declare_dram_parameter("q0", [2, D, S], bf, isOutput=False)
    qr_d = nc.declare_dram_parameter("qr", [3, D, S], bf, isOutput=False)
    kt_d = nc.declare_dram_parameter("kt", [2, D, S], bf, isOutput=False)
    v_d = nc.declare_dram_parameter("v", [D, NB, 129], bf, isOutput=False)
    cs_d = nc.declare_dram_parameter("cs", [2, D, S], bf, isOutput=False)
    msk_d = nc.declare_dram_parameter("msk", [D, 3, 128], bf, isOutput=False)
    out_d = nc.declare_dram_parameter("out", [HQ_PER_CORE, NB, D, 128], bf, isOutput=True)

    with tile.TileContext(nc) as tc:
        with (
            tc.tile_pool(name="persist", bufs=1) as persist,
            tc.tile_pool(name="probs", bufs=3) as probs_pool,
            tc.tile_pool(name="norm", bufs=3) as norm_pool,
            tc.tile_pool(name="osb", bufs=3) as osb_pool,
            tc.tile_pool(name="grp", bufs=2, space="PSUM") as grp_psum,
            tc.tile_pool(name="acc", bufs=2, space="PSUM") as acc_psum,
        ):
            qt0 = persist.tile([D, 2, S], bf)
            qr = persist.tile([D, 3, S], bf)
            qsw = persist.tile([D, 3, S], bf)  # DVE-swapped q heads 1-3
            kt = persist.tile([D, 2, S], bf)
            vt = persist.tile([D, NB, 129], bf)
            cs = persist.tile([D, 2, S], bf)
            msk = persist.tile([D, 3, 128], bf)
            ones = persist.tile([D, 128], bf)
            ropek = persist.tile([D, S], bf)
            ropeq = persist.tile([D, HQ_PER_CORE, S], bf)
            ropet = persist.tile([D, 2, S], bf)

            # Host ships straight + half-swapped copies of q/k; chunk
            # boundaries match piece needs so head-0 attention starts as
            # early as possible while later heads stream in.
            ktr = kt_d.rearrange("s d t -> d s t")
            csr = cs_d.rearrange("s d t -> d s t")
            q0r = q0_d.rearrange("s d t -> d s t")
            # Chunked so head-0 rope starts early; q heads 1-3 ship single
            # copy (their RoPE half-swap happens on DVE) and land before the
            # head-0 tail chunks they do not gate.
            chunks = [(0, 384), (384, 768), (768, 1152), (1152, 1536),
                      (1536, 1920), (1920, S)]
            for ci, (lo, hi) in enumerate(chunks):
                nc.sync.dma_start(out=kt[:, :, lo:hi], in_=ktr[:, :, lo:hi])
                nc.sync.dma_start(out=cs[:, :, lo:hi], in_=csr[:, :, lo:hi])
                nc.sync.dma_start(out=qt0[:, :, lo:hi], in_=q0r[:, :, lo:hi])
                if ci == 0:
                    nc.sync.dma_start(out=msk, in_=msk_d[:])
                elif ci == 1:
                    nc.sync.dma_start(out=vt, in_=v_d[:])
                elif ci == 3:
                    nc.sync.dma_start(out=qr[:, 0], in_=qr_d[0])
                elif ci == 5:
                    nc.sync.dma_start(out=qr[:, 1], in_=qr_d[1])
                    nc.sync.dma_start(out=qr[:, 2], in_=qr_d[2])
            nc.vector.memset(ones, 1.0)

            # trigger the exp ACT-table load early (off the critical path)
            tldw = norm_pool.tile([D, 3, 1], f32, tag="rz")
            nc.scalar.activation(tldw[:, 0], ones[:, :1], EXP)

            # PE warm-up: DMA-independent dummy matmuls (ones x ones) keep
            # the HAM activity window busy so the real stream starts ramped.
            wz = acc_psum.tile([D, 512], f32, tag="ot")
            for _ in range(34):
                nc.tensor.matmul(
                    wz[:, :128], lhsT=ones, rhs=ones, start=True, stop=True
                )

            def rope_k(lo, hi):
                sl = slice(lo, hi)
                nc.vector.tensor_mul(ropek[:, sl], kt[:, 0, sl], cs[:, 0, sl])
                nc.vector.tensor_mul(ropet[:, 0, sl], kt[:, 1, sl], cs[:, 1, sl])
                nc.vector.tensor_add(ropek[:, sl], ropek[:, sl], ropet[:, 0, sl])

            def rope_q(h, lo, hi):
                sl = slice(lo, hi)
                straight = qt0[:, 0, sl] if h == 0 else qr[:, h - 1, sl]
                swapped = qt0[:, 1, sl] if h == 0 else qsw[:, h - 1, sl]
                nc.vector.tensor_mul(ropeq[:, h, sl], straight, cs[:, 0, sl])
                nc.vector.tensor_mul(ropet[:, 1, sl], swapped, cs[:, 1, sl])
                nc.vector.tensor_add(
                    ropeq[:, h, sl], ropeq[:, h, sl], ropet[:, 1, sl]
                )

            def swap_q(h):
                # full-span partition-offset copies (4x single-src DVE mode)
                nc.vector.tensor_copy(qsw[0:64, h - 1], qr[64:128, h - 1])
                nc.vector.tensor_copy(qsw[64:128, h - 1], qr[0:64, h - 1])

            def emit_qk(h, st, gp):
                """QK + additive-mask matmuls for one strip into the group
                tile at st['goff']."""
                span = st["qhi"] - st["qlo"]
                go = st["goff"]
                masks = []
                if st["diag_u"] is not None:
                    masks.append((st["diag_u"], 0))
                if st["i2_u"] is not None:
                    masks.append((st["i2_u"], 1))
                nc.tensor.matmul(
                    gp[:, go : go + span],
                    lhsT=ropek[:, st["kvblk"] * 128 : (st["kvblk"] + 1) * 128],
                    rhs=ropeq[:, h, st["qlo"] : st["qhi"]],
                    start=True,
                    stop=not masks,
                )
                for mi, (u, g) in enumerate(masks):
                    nc.tensor.matmul(
                        gp[:, go + u : go + u + 128],
                        lhsT=msk[:, 2],
                        rhs=msk[:, g],
                        start=False,
                        stop=mi == len(masks) - 1,
                    )

            def emit_piece_back(work):
                """PV+Z matmuls for all strips of a piece, then normalize +
                store. otq layout: q-block b of the piece at cols
                [129b, 129b+129); col 129b+128 accumulates Z."""
                h, b0, b1, order, pbg, otq = work
                nq = b1 - b0 + 1
                last_for_qb = {}
                for si, st in enumerate(order):
                    for qb in range(st["qlo"] // 128, st["qhi"] // 128):
                        last_for_qb[qb] = si
                # start=True ONLY on the very first matmul into the bank: it
                # clears has_written for the WHOLE bank, so later first-
                # writes per q-block chain must use start=False (overwrite-
                # where-bit-unset initializes them correctly).
                for si, st in enumerate(order):
                    for k in range((st["qhi"] - st["qlo"]) // 128):
                        qb = st["qlo"] // 128 + k
                        b = qb - b0
                        nc.tensor.matmul(
                            otq[:, b],
                            lhsT=pbg[:, st["goff"] + 128 * k : st["goff"] + 128 * (k + 1)],
                            rhs=vt[:, st["kvblk"]],
                            start=si == 0 and k == 0,
                            stop=last_for_qb[qb] == si,
                            skip_group_check=True,
                        )
                # normalize: strided recip of the nq Z columns, then one
                # dv-broadcast multiply; store bf16.
                rzt = norm_pool.tile([D, 3, 1], f32, tag="rz")
                nc.vector.reciprocal_approx_fast(
                    rzt[:, :nq], otq[:, :nq, 128:129]
                )
                osb = osb_pool.tile([D, 3, 128], bf, tag="osb")
                nc.vector.tensor_mul(
                    osb[:, :nq],
                    otq[:, :nq, :128],
                    rzt[:, :nq].broadcast_to([D, nq, 128]),
                )
                orh = out_d[h].rearrange("j p v -> p j v")
                nc.sync.dma_start(out=orh[:, b0 : b0 + nq], in_=osb[:, :nq])

            # Software-pipelined emission with a lag: PE runs QK of later
            # pieces while ACT computes earlier pieces' exps. Units are
            # ordered by input-DMA arrival (head-0 chunks first, head 1
            # interleaved as its q lands, heads 2/3 after) so the exp
            # stream runs dense from the first piece instead of pacing on
            # head-0's chunk arrivals. Rope/swap ops are emitted just in
            # time, keeping the DVE queue free of long head-of-line waits.
            from collections import deque

            order = [(h, p) for h in range(HQ_PER_CORE) for p in range(6)]
            ropeq_done = [0] * HQ_PER_CORE
            ropek_done = 0
            swapped = set()
            pending = deque()
            for ui, (h, pidx) in enumerate(order):
                b0, b1 = PIECES[pidx]
                need = (b1 + 1) * 128
                if h >= 1 and h not in swapped:
                    swap_q(h)
                    swapped.add(h)
                if ropek_done < need:
                    rope_k(ropek_done, need)
                    ropek_done = need
                if ropeq_done[h] < need:
                    rope_q(h, ropeq_done[h], need)
                    ropeq_done[h] = need
                LAG = 1 if ui < 4 else 2
                strips, gcols = _pack_group(_strips_for_piece(b0, b1))
                gp = grp_psum.tile([D, 1536], f32, tag="gp")
                pbg = probs_pool.tile([D, 1536], bf, tag="pb")
                otq = acc_psum.tile([D, 3, 129], f32, tag="ot")
                for st in strips:
                    emit_qk(h, st, gp)
                nc.scalar.activation(
                    pbg[:, :gcols], gp[:, :gcols], EXP, scale=SCALE
                )
                pending.append((h, b0, b1, strips, pbg, otq))
                while len(pending) > LAG:
                    emit_piece_back(pending.popleft())
            while pending:
                emit_piece_back(pending.popleft())

    nc.finalize()
    return nc


@lru_cache(maxsize=1)
def _rope_tables():
    inv_freq = 1.0 / (ROPE_BASE ** (np.arange(0, D, 2, dtype=np.float64) / D))
    pos = np.arange(S, dtype=np.float64)
    freqs = pos[:, None] * inv_freq[None, :]  # [S, 64]
    emb = np.concatenate([freqs, freqs], axis=-1)  # [S, D]
    # match the f32 reference: compute cos/sin at f32 granularity
    cosT = np.cos(emb.astype(np.float32)).T.astype(np.float32)  # [D, S]
    sinT = np.sin(emb.astype(np.float32)).T.astype(np.float32)
    sinTpm = np.concatenate([-sinT[:64], sinT[64:]], axis=0)
    return cosT, sinTpm


def _mask_tiles():
    """[128, 3, 128]: additive score masks (0 keep / -1e30 drop) for the
    causal-diag and window-tail blocks, plus a 128x128 identity (the
    stationary operand of the mask-accumulate matmuls)."""
    c = np.arange(128)[:, None]
    u = np.arange(128)[None, :]
    a_diag = np.where(u >= c, 0.0, -1e30).astype(np.float32)
    a_tail = np.where(u <= c, 0.0, -1e30).astype(np.float32)
    ident = np.eye(128, dtype=np.float32)
    return np.stack([a_diag, a_tail, ident], axis=1)  # [128, 3, 128]


def _swap_halves(xT):
    return np.concatenate([xT[64:], xT[:64]], axis=0)


def _install_ntff_shim():
    """Provide antenv.axon_hooks (NTFF profile hook) if the image lacks it,
    so run_bass_kernel_spmd(trace=True) can capture HW profiles via the
    axon PJRT .so. Silently no-ops if unavailable."""
    import sys
    import types

    try:
        from antenv.axon_hooks import get_axon_ntff_profile_hook  # noqa: F401

        return
    except ImportError:
        pass
    try:
        import contextlib
        import ctypes

        lib = ctypes.CDLL("/opt/axon/libaxon_pjrt.so")
        if not hasattr(lib, "axon_start_nrt_profile"):
            return
        lib.axon_start_nrt_profile.argtypes = [
            ctypes.POINTER(ctypes.c_int64),
            ctypes.c_size_t,
        ]
        lib.axon_start_nrt_profile.restype = ctypes.c_int64
        lib.axon_stop_nrt_profile.argtypes = [ctypes.c_char_p]
        lib.axon_stop_nrt_profile.restype = ctypes.c_int64

        @contextlib.contextmanager
        def _hook(output_dir, device_ids):
            import jax

            jax.devices()
            if device_ids:
                ids = (ctypes.c_int64 * len(device_ids))(*device_ids)
                rc = lib.axon_start_nrt_profile(ids, len(device_ids))
            else:
                rc = lib.axon_start_nrt_profile(None, 0)
            if rc != 0:
                raise RuntimeError(f"axon_start_nrt_profile rc={rc}")
            try:
                yield
            finally:
                n = lib.axon_stop_nrt_profile(str(output_dir).encode())
                print(f"ntff profile: {n} file(s) -> {output_dir}", file=sys.stderr)

        mod = types.ModuleType("antenv.axon_hooks")
        mod._hook = _hook
        mod.get_axon_ntff_profile_hook = lambda: _hook
        mod.set_axon_ntff_profile_hook = lambda h: setattr(mod, "_hook", h)
        import antenv

        antenv.axon_hooks = mod
        sys.modules["antenv.axon_hooks"] = mod
    except Exception:
        pass


def kernel(query_states, key_states, value_states):
    from concourse.bass_utils import run_bass_kernel_spmd

    _install_ntff_shim()

    nc = _build_program()

    q = np.asarray(query_states)[0]  # [S, 4096]
    k = np.asarray(key_states)[0]  # [S, 1024]
    v = np.asarray(value_states)[0]  # [S, 1024]

    cosT, sinTpm = _rope_tables()
    cs = np.stack([cosT, sinTpm], axis=0).astype(BF16)  # [2, D, S]
    msk = _mask_tiles().astype(BF16)

    in_maps = []
    for c in range(N_CORES):
        q0h = np.ascontiguousarray(q[:, 4 * c * D : (4 * c + 1) * D].T)  # [D, S]
        q0 = np.stack([q0h, _swap_halves(q0h)], axis=0).astype(BF16)
        qr = np.empty((3, D, S), dtype=BF16)
        for hh in range(1, HQ_PER_CORE):
            h = 4 * c + hh
            qr[hh - 1] = np.ascontiguousarray(q[:, h * D : (h + 1) * D].T).astype(BF16)
        kh = np.ascontiguousarray(k[:, c * D : (c + 1) * D].T)
        kt = np.stack([kh, _swap_halves(kh)], axis=0).astype(BF16)
        vh = v[:, c * D : (c + 1) * D]  # [S, D]
        vts = np.ones((D, NB, 129), dtype=BF16)
        vts[:, :, :128] = vh.reshape(NB, 128, D).transpose(1, 0, 2).astype(BF16)
        in_maps.append({"q0": q0, "qr": qr, "kt": kt, "v": vts, "cs": cs, "msk": msk})

    res = run_bass_kernel_spmd(nc, in_maps, core_ids=list(range(N_CORES)))
    global LAST_RESULT
    LAST_RESULT = res

    out = np.empty((S, 32, D), dtype=np.float32)
    for c in range(N_CORES):
        o = np.asarray(res.results[c]["out"], dtype=np.float32)  # [4, NB, D, 128]
        for hh in range(HQ_PER_CORE):
            out[:, 4 * c + hh, :] = o[hh].reshape(S, D)
    return out.reshape(1, S, 32 * D)
